# revision 13
# baseline (speedup 1.0000x reference)
"""Trainium2 Bass kernel for nn_Attention_1 (B=32, T=2048, H=1024, D_OUT=128).

Math: score = (hs @ W_score) @ h_t is reassociated as hs @ (W_score @ h_t),
turning the dominant [B*T,H]@[H,H] matmul into a per-sample matvec. The
kernel streams each core's 4 samples of hidden_states (32 MB) through SBUF
exactly once, cast to fp16 on the fly by the DMA engines.

Per tile: one fused DVE tensor_tensor_reduce computes hs*v and accumulates
the score column; the context vector streams the same fp16 tiles through
the PE with the softmax weights as the stationary operand. Softmax
cross-partition reductions run on PE (transpose + ones-matmul broadcast)
to keep the gpsimd queue free for the cast-DMA stream.

Sharding: data-parallel over batch, 4 samples per core across 8 cores.
"""

import numpy as np
from contextlib import ExitStack

import concourse.bass as bass
import concourse.bacc as bacc
import concourse.mybir as mybir
from concourse import tile
from concourse import bass_utils
from concourse.masks import make_identity
from concourse import bass_isa

F32 = mybir.dt.float32
F16 = mybir.dt.float16
B, T, H, DOUT = 32, 2048, 1024, 128
NCORES = 8
BL = B // NCORES     # 4 samples per core
P = 128
NT = T // P          # 16 t-tiles per sample
NG = 4               # DMA groups per sample
TJ = NT // NG        # 4 t-tiles per group (2 MB per DMA)
NH = H // P          # 8 h-chunks
NPA = 2 * H // P     # 16 k-chunks of pre-activation
LAND_BUFS = 6        # fp32 landing slots [128, 4, 1024]


def _emit(ctx: ExitStack, tc: "tile.TileContext", hs_d, ws_d, wo_d, out_d, scr_d):
    nc = tc.nc

    const = ctx.enter_context(tc.tile_pool(name="const", bufs=1))
    wtp = ctx.enter_context(tc.tile_pool(name="wtp", bufs=1))
    wnp = ctx.enter_context(tc.tile_pool(name="wnp", bufs=4))
    hsp = ctx.enter_context(tc.tile_pool(name="hsp", bufs=LAND_BUFS))
    wrk = ctx.enter_context(tc.tile_pool(name="wrk", bufs=2))
    sml = ctx.enter_context(tc.tile_pool(name="sml", bufs=2))
    ps = ctx.enter_context(tc.tile_pool(name="ps", bufs=1, space="PSUM"))

    identity = const.tile([P, P], F32, tag="ident")
    make_identity(nc, identity[:])
    ones_row = const.tile([1, P], F32, tag="ones")
    nc.gpsimd.memset(ones_row[:], 1.0)

    # W_out [2048,128] -> wo[k_part, c, n]  (single rearranged DMA)
    wo = const.tile([P, NPA, DOUT], F32, tag="wo")
    nc.sync.dma_start(wo[:], wo_d.rearrange("(c p) n -> p c n", p=P))

    # h_t columns: ht[p, c, b] = hs[b, T-1, c*128+p]
    ht = const.tile([P, NH, BL], F32, tag="ht")
    for b in range(BL):
        nc.sync.dma_start(
            ht[:, :, b], hs_d[b, T - 1, :].rearrange("(c p) -> p c", p=P)
        )

    # Transpose W_score via PE: wts[kc][k_part, h_free] = W_score[h, kc*128+k]
    wts = [wtp.tile([P, H], F32, tag=f"wt{kc}", name=f"wt{kc}") for kc in range(NH)]
    for hg in range(2):
        wng = []
        for j in range(4):
            wn = wnp.tile([P, H], F32, tag="wn")
            nc.sync.dma_start(wn[:], ws_d[(hg * 4 + j) * P:(hg * 4 + j + 1) * P, :])
            wng.append(wn)
        for kc in range(NH):
            tps = ps.tile([P, 4, P], F32, tag="big", bufs=2)
            for j in range(4):
                nc.tensor.transpose(
                    tps[:, j, :], wng[j][:, kc * P:(kc + 1) * P], identity[:]
                )
            nc.vector.tensor_copy(wts[kc][:, hg * 512:(hg + 1) * 512], tps[:])

    # v[b, :] = W_score @ h_t[b]  -> [BL, H] (rows)
    v_sb = const.tile([BL, H], F32, tag="vsb")
    for n2 in range(2):
        pv = ps.tile([BL, 512], F32, tag="acc", bufs=1)
        for kc in range(NH):
            nc.tensor.matmul(
                pv[:], ht[:, kc, :], wts[kc][:, n2 * 512:(n2 + 1) * 512],
                start=(kc == 0), stop=(kc == NH - 1),
            )
        nc.vector.tensor_copy(v_sb[:, n2 * 512:(n2 + 1) * 512], pv[:])

    # vb16[p, b, h] = v[b, h] broadcast across partitions (gpsimd)
    vb16 = const.tile([P, BL, H], F32, tag="vb16")
    for b in range(BL):
        vrow = wrk.tile([1, H], F32, tag="vrow", bufs=1)
        nc.scalar.dma_start(vrow[:], v_sb[b:b + 1, :])
        nc.gpsimd.partition_broadcast(vb16[:, b, :], vrow[:])

    pa = const.tile([P, NPA, BL], F32, tag="pa")

    for b in range(BL):
        # fp16 landing; per partition p, group g holds t = g*512 + p*4 + j
        score = sml.tile([P, NT], F32, tag="score")
        lands = []
        for g in range(NG):
            land = hsp.tile([P, TJ, H], F32, tag="land")
            nc.sync.dma_start(
                land[:],
                hs_d[b, g * 512:(g + 1) * 512, :].rearrange(
                    "(p j) h -> p j h", p=P
                ),
            )
            lands.append(land)
            for j in range(TJ):
                ti = g * TJ + j
                prod = wrk.tile([P, H], F32, tag="prod")
                nc.vector.tensor_tensor(
                    out=prod[:], in0=land[:, j, :], in1=vb16[:, b, :],
                    op=mybir.AluOpType.mult,
                )
                dum = wrk.tile([P, H], F16, tag="dum")
                nc.scalar.activation(
                    dum[:], prod[:], mybir.ActivationFunctionType.Copy,
                    accum_out=score[:, ti:ti + 1],
                )

        # numerically-stable softmax; partition reductions via PE transpose
        m1 = sml.tile([P, 1], F32, tag="m1")
        nc.vector.tensor_reduce(
            m1[:], score[:], axis=mybir.AxisListType.X, op=mybir.AluOpType.max
        )
        gma = sml.tile([P, 1], F32, tag="gma")
        nc.gpsimd.partition_all_reduce(
            gma[:], m1[:], channels=P, reduce_op=bass_isa.ReduceOp.max
        )
        gmn = sml.tile([P, 1], F32, tag="gmn")
        nc.scalar.mul(gmn[:], gma[:], -1.0)

        e = sml.tile([P, NT], F32, tag="e")
        ssum = sml.tile([P, 1], F32, tag="ssum")
        nc.scalar.activation(
            e[:], score[:], mybir.ActivationFunctionType.Exp,
            bias=gmn[:], scale=1.0, accum_out=ssum[:],
        )
        sa = sml.tile([P, 1], F32, tag="sa")
        nc.gpsimd.partition_all_reduce(
            sa[:], ssum[:], channels=P, reduce_op=bass_isa.ReduceOp.add
        )
        reca = sml.tile([P, 1], F32, tag="reca")
        nc.vector.reciprocal(reca[:], sa[:])
        w16 = sml.tile([P, NT], F32, tag="w16")
        nc.vector.tensor_scalar_mul(w16[:], e[:], reca[:])

        # context row: ctx[h] = sum_t w16[t] * hs16[t, h] on PE
        cr0 = ps.tile([1, 512], F32, tag="big", bufs=2)
        cr1 = ps.tile([1, 512], F32, tag="big", bufs=2)
        for ti in range(NT):
            land = lands[ti // TJ]
            j = ti % TJ
            lw = w16[:, ti:ti + 1]
            nc.tensor.matmul(
                cr0[:], lw, lands[ti // TJ][:, j, 0:512],
                start=(ti == 0), stop=(ti == NT - 1),
            )
            nc.tensor.matmul(
                cr1[:], lw, lands[ti // TJ][:, j, 512:H],
                start=(ti == 0), stop=(ti == NT - 1),
            )
        ctxrow = sml.tile([1, H], F32, tag="ctxrow")
        nc.vector.tensor_copy(ctxrow[:, 0:512], cr0[:])
        nc.vector.tensor_copy(ctxrow[:, 512:H], cr1[:])

        # columnize ctx into pa via a DRAM bounce
        nc.sync.dma_start(scr_d[b:b + 1, :], ctxrow[0:1, :])
        nc.sync.dma_start(
            pa[:, 0:NH, b], scr_d[b, :].rearrange("(c p) -> p c", p=P)
        )
        nc.vector.tensor_copy(pa[:, NH:NPA, b], ht[:, :, b])

    # columnize ctx rows via DRAM bounce reads (after the land stream)
    for b in range(BL):
        nc.sync.dma_start(
            pa[:, 0:NH, b], scr_d[b, :].rearrange("(c p) -> p c", p=P)
        )

    # attention_vector = tanh(pre_act @ W_out), batched over all samples
    fin = ps.tile([BL, DOUT], F32, tag="acc", bufs=1)
    for c in range(NPA):
        nc.tensor.matmul(
            fin[:], pa[:, c, :], wo[:, c, :],
            start=(c == 0), stop=(c == NPA - 1),
        )
    res = sml.tile([BL, DOUT], F32, tag="res")
    nc.scalar.activation(res[:], fin[:], mybir.ActivationFunctionType.Tanh)
    nc.sync.dma_start(out_d[:, :], res[:])


_CACHE = None


def build():
    global _CACHE
    if _CACHE is None:
        nc = bacc.Bacc(
            "TRN2", target_bir_lowering=False, debug=False, num_devices=NCORES
        )
        hs_d = nc.dram_tensor("hs", [BL, T, H], F32, kind="ExternalInput").ap()
        ws_d = nc.dram_tensor("w_score", [H, H], F32, kind="ExternalInput").ap()
        wo_d = nc.dram_tensor("w_out", [2 * H, DOUT], F32, kind="ExternalInput").ap()
        out_d = nc.dram_tensor("out", [BL, DOUT], F32, kind="ExternalOutput").ap()
        scr_d = nc.dram_tensor("scratch", [BL, H], F32, kind="ExternalOutput").ap()
        with tile.TileContext(nc) as tc:
            with ExitStack() as ctx:
                _emit(ctx, tc, hs_d, ws_d, wo_d, out_d, scr_d)
        nc.compile()
        _CACHE = nc
    return _CACHE


def make_in_maps(hidden_states, W_score, W_out):
    hs = np.ascontiguousarray(np.asarray(hidden_states, dtype=np.float32))
    ws = np.ascontiguousarray(np.asarray(W_score, dtype=np.float32))
    wo = np.ascontiguousarray(np.asarray(W_out, dtype=np.float32))
    return [
        {"hs": hs[c * BL:(c + 1) * BL], "w_score": ws, "w_out": wo}
        for c in range(NCORES)
    ]


def kernel(hidden_states, W_score, W_out):
    nc = build()
    in_maps = make_in_maps(hidden_states, W_score, W_out)
    res = bass_utils.run_bass_kernel_spmd(nc, in_maps, core_ids=list(range(NCORES)))
    return np.concatenate([r["out"] for r in res.results], axis=0)


if __name__ == "__main__":
    import jax

    with jax.default_device(jax.devices("cpu")[0]):
        key = jax.random.key(0)
        k1, k2, k3 = jax.random.split(key, 3)
        hs = np.asarray(jax.random.normal(k1, (B, T, H), dtype=np.float32))
    out = kernel(hs, np.eye(H, dtype=np.float32), np.ones((2 * H, DOUT), np.float32))
    print(out.shape, out.dtype)
